# revision 35
# baseline (speedup 1.0000x reference)
"""Trainium2 Bass kernel for nn_CCIM (dot-product intervention / CCIM block).

Reference computation (B=1024, K=256, D=1024, P=768):
    q = jf @ Wq                      [B, P]
    k = conf @ Wk                    [K, P]
    s = (q @ k.T) / 32               [B, K]
    a = softmax(s, axis=-1)          [B, K]
    out = jf + a @ (conf * prior)    [B, D]

Distribution: data-parallel over B across 8 NeuronCores (128 rows each);
weights/confounders replicated on every core; no collectives.

Matmuls run in bf16 with fp32 PSUM accumulation; jf stays exact fp32 through
to the final add, so bf16 rounding only perturbs the small g_z correction
term (measured rel-L2 vs the fp32 reference: 2.7e-4). The host pre-casts
Wq/Wk to bf16 and pre-computes conf.T (k-chain operand) and conf*prior
(g_z operand) - numerically equivalent to on-device casting/transposing but
saves HBM traffic, all 16 conf PE-transposes, and the prior multiply.

Per-core schedule (engineered against the neuron-profile trace):
  - ~4.5us of dummy matmuls first flip the PE HAM clock-gate to 2.4 GHz.
  - DMA rings are dependency-isolated: Sync ring carries jf then the 8 Wq
    chunks then conf*prior; Scalar ring carries the 8 Wk chunks; the GpSimd
    (SWDGE) ring carries conf.T (its 512B-row packets would otherwise queue
    ahead of Wq). Rings drain in instruction order, so order = priority.
  - jf.T is built with bf16 PE transposes (xbar DMA-transpose serializes all
    DMA traffic and is avoided).
  - kT (=(conf@Wk).T) and qT (=(jf@Wq).T) accumulate over D-chunks with their
    matmuls interleaved per chunk (the PE stream executes strictly in order,
    so phase-ordered emission would head-of-line block).
  - PSUM sub-tiles pack 2-4 accumulation groups per bank: the bank's single
    start=True matmul clears the whole bank's has_written bits, later groups
    first-write with start=False, ordered via add_dep_helper.
  - scores = qT.T @ kT per P-tile; exp(s/32) with fused row-sum on ACT (no
    max-subtraction needed: |s|/32 < ~6); E.T via PE transpose; gz = E.T-
    tiles @ (conf*prior); out = gz*(1/denom) + jf fused on DVE, 4-way split
    and pipelined with the two output DMAs.
"""

import numpy as np

B, K, D, P = 1024, 256, 1024, 768
N_CORES = 8
BS = B // N_CORES  # 128 rows per core

_COMPILED = {}


def _build():
    import concourse.mybir as mybir
    import concourse.tile as tile
    from concourse import bacc
    from concourse.tile_rust import add_dep_helper
    from concourse.compiler_utils import get_compiler_flags, set_compiler_flags
    from concourse.masks import make_identity

    saved_flags = get_compiler_flags()
    if saved_flags:
        set_compiler_flags(
            [
                f.replace("--enable-ldw-opt=false", "--enable-ldw-opt=true")
                for f in saved_flags
            ]
        )

    F32 = mybir.dt.float32
    BF = mybir.dt.bfloat16
    KD = D // 128  # 8 contraction tiles over D
    MP = P // 128  # 6 partition tiles over P
    KT = K // 128  # 2 tiles over K

    nc = bacc.Bacc(
        "TRN2",
        target_bir_lowering=False,
        debug=False,
        num_devices=N_CORES,
    )

    jf = nc.dram_tensor("jf", [BS, D], F32, kind="ExternalInput")
    conft = nc.dram_tensor("conft", [D, K], BF, kind="ExternalInput")
    confp = nc.dram_tensor("confp", [K, D], BF, kind="ExternalInput")
    wq = nc.dram_tensor("wq", [D, P], BF, kind="ExternalInput")
    wk = nc.dram_tensor("wk", [D, P], BF, kind="ExternalInput")
    out = nc.dram_tensor("out", [BS, D], F32, kind="ExternalOutput")

    with tile.TileContext(nc) as tc:
        with (
            tc.tile_pool(name="cst", bufs=1) as cst,
            tc.tile_pool(name="per", bufs=1) as per,
            tc.tile_pool(name="wqp", bufs=1) as wqp,
            tc.tile_pool(name="wkp", bufs=1) as wkp,
            tc.tile_pool(name="ps", bufs=6, space="PSUM") as ps,
            tc.tile_pool(name="pst", bufs=2, space="PSUM") as pst,
        ):
            # Identity first: its gpsimd memset/affine must precede the 16
            # SWDGE trigger instructions on the GpSimd queue, or it lands
            # ~13us late and stalls every transpose behind it.
            ident = cst.tile([128, 128], F32, tag="ident", name="ident")
            make_identity(nc, ident[:])
            ident_bf = cst.tile([128, 128], BF, tag="ident_bf", name="ident_bf")
            nc.vector.tensor_copy(ident_bf[:], ident[:])

            psw = ps.tile([BS, 512], F32, tag="bank", name="psw")
            # PE warmup: ~3us of dummy matmuls to flip the HAM clock-gate to
            # 2.4 GHz before real matmuls arrive (psw's first real use also
            # has start=True, so contents don't matter).
            with nc.named_scope("warmup"):
                for _ in range(42):
                    nc.tensor.matmul(
                        psw[:, 0:128], lhsT=ident_bf[:], rhs=ident_bf[:],
                        start=True, stop=True,
                    )

            wqt2 = [
                wqp.tile([128, 2 * P], BF, tag=f"wq{g}", name=f"wq{g}")
                for g in range(KD // 2)
            ]
            wqt = [wqt2[kk // 2][:, P * (kk % 2) : P * (kk % 2 + 1)] for kk in range(KD)]
            wkt2 = [
                wkp.tile([128, 2 * P], BF, tag=f"wk{g}", name=f"wk{g}")
                for g in range(KD // 2)
            ]
            wkt = [wkt2[kk // 2][:, P * (kk % 2) : P * (kk % 2 + 1)] for kk in range(KD)]
            # Wk chunks on the Scalar HWDGE ring (its own ring so Wk traffic
            # never queues behind jf/conf).
            for kk in range(KD):
                nc.scalar.dma_start(
                    out=wkt[kk], in_=wk.ap()[128 * kk : 128 * (kk + 1), :]
                )

            # ---- input DMAs (jf/conf/prior first, then Wq stream, Wk stream)
            jf_sb = per.tile([BS, D], F32, tag="jf", name="jf")
            nc.sync.dma_start(out=jf_sb[:], in_=jf.ap())
            # conf.T comes pre-transposed from the host: one DMA into 8
            # column-blocks [128, K] each (kills 16 PE transposes + copies).
            confT_big = per.tile([128, KD * K], BF, tag="confT", name="confT")
            nc.gpsimd.dma_start(
                out=confT_big[:].rearrange("p (kk c) -> p kk c", kk=KD),
                in_=conft.ap().rearrange("(kk p) c -> p kk c", p=128),
            )
            confT = [confT_big[:, K * kk : K * (kk + 1)] for kk in range(KD)]
            # Wq chunks on the Sync ring, after jf/confT.
            for kk in range(KD):
                nc.sync.dma_start(
                    out=wqt[kk], in_=wq.ap()[128 * kk : 128 * (kk + 1), :]
                )
            # conf*prior (host-folded) for the gz matmul; only needed at the
            # tail, so it loads after the Wq stream.
            confp_sb = per.tile([128, KT * D], BF, tag="confp", name="confp")
            nc.sync.dma_start(
                out=confp_sb[:].rearrange("p (t c) -> p t c", t=KT),
                in_=confp.ap().rearrange("(t p) c -> p t c", p=128),
            )



            # ---- jfT: bf16 PE transposes (1 cyc/row) from jf_bf
            jf_bf = per.tile([BS, D], BF, tag="jf_bf", name="jf_bf")
            jfT_t = [
                per.tile([128, BS], BF, tag=f"jfT{kk}", name=f"jfT{kk}")
                for kk in range(KD)
            ]
            with nc.named_scope("jfT"):
                nc.vector.tensor_copy(jf_bf[:], jf_sb[:])
                for kk in range(KD):
                    pt = pst.tile([128, 256], BF, tag="pc", name="pt")
                    nc.tensor.transpose(
                        pt[:, 0:128], jf_bf[:, 128 * kk : 128 * (kk + 1)], ident_bf[:]
                    )
                    nc.vector.tensor_copy(jfT_t[kk][:], pt[:, 0:128])
            jfT = [t[:] for t in jfT_t]

            # ---- kT and qT matmuls, interleaved per D-chunk so the PE
            # stream (strict in-order) never head-of-line blocks. qT is
            # computed directly (lhsT=Wq tile, rhs=jfT) so no q transpose
            # chain is needed at the end. Both pack 2 sub-tiles per PSUM
            # bank with an ordered first-write (see kT comment).
            psk = [
                ps.tile([128, 2 * K], F32, tag="bank", name=f"psk{i}")
                for i in range(MP // 2)
            ]
            psqt = [
                ps.tile([128, 4 * BS], F32, tag="bank", name="psqt0"),
                ps.tile([128, 2 * BS], F32, tag="bank", name="psqt1"),
            ]

            def psk_ap(mm):
                return psk[mm // 2][:, K * (mm % 2) : K * (mm % 2 + 1)]

            def psqt_ap(pp):
                b, j = (0, pp) if pp < 4 else (1, pp - 4)
                return psqt[b][:, BS * j : BS * (j + 1)]

            bank_opener = {}
            qt_opener = {}
            with nc.named_scope("qk_mm"):
                for kk in range(KD):
                    for mm in range(MP):
                        inst = nc.tensor.matmul(
                            psk_ap(mm),
                            lhsT=wkt[kk][:, 128 * mm : 128 * (mm + 1)],
                            rhs=confT[kk],
                            start=(kk == 0 and mm % 2 == 0),
                            stop=(kk == KD - 1),
                        )
                        if kk == 0:
                            b = mm // 2
                            if mm % 2 == 0:
                                bank_opener[b] = inst
                            else:
                                add_dep_helper(
                                    inst.ins,
                                    bank_opener[b].ins,
                                    sync=False,
                                    reason="first-write waits on bank open",
                                )
                    for pp in range(MP):
                        b, j = (0, pp) if pp < 4 else (1, pp - 4)
                        inst = nc.tensor.matmul(
                            psqt_ap(pp),
                            lhsT=wqt[kk][:, 128 * pp : 128 * (pp + 1)],
                            rhs=jfT[kk],
                            start=(kk == 0 and j == 0),
                            stop=(kk == KD - 1),
                        )
                        if kk == 0:
                            if j == 0:
                                qt_opener[b] = inst
                            else:
                                add_dep_helper(
                                    inst.ins,
                                    qt_opener[b].ins,
                                    sync=False,
                                    reason="first-write waits on bank open",
                                )

            # ---- qT: psum -> bf16 copies (one per packed bank)
            qT3 = [
                per.tile([128, 4 * BS], BF, tag="qT0", name="qT0"),
                per.tile([128, 2 * BS], BF, tag="qT1", name="qT1"),
            ]
            with nc.named_scope("qT"):
                for b in range(2):
                    nc.vector.tensor_copy(qT3[b][:], psqt[b][:])
            qT = [
                qT3[0][:, BS * pp : BS * (pp + 1)] if pp < 4
                else qT3[1][:, BS * (pp - 4) : BS * (pp - 3)]
                for pp in range(MP)
            ]

            kT3 = [
                per.tile([128, 2 * K], BF, tag=f"kT{b}", name=f"kT{b}")
                for b in range(MP // 2)
            ]
            with nc.named_scope("kT_copy"):
                for b in range(MP // 2):
                    nc.vector.tensor_copy(kT3[b][:], psk[b][:])
            kT = [kT3[mm // 2][:, K * (mm % 2) : K * (mm % 2 + 1)] for mm in range(MP)]

            # ---- scores = q @ k.T (accumulate over P tiles)
            ps_s = ps.tile([BS, 512], F32, tag="bank", name="ps_s")[:, 0:K]
            with nc.named_scope("scores"):
                for pp in range(MP):
                    nc.tensor.matmul(
                        ps_s[:],
                        lhsT=qT[pp],
                        rhs=kT[pp],
                        start=(pp == 0),
                        stop=(pp == MP - 1),
                    )

            # ---- softmax numerator + denominator (no max-subtraction)
            E_sb = per.tile([BS, K], BF, tag="E", name="E")
            denom = per.tile([BS, 1], F32, tag="denom", name="denom")
            r_sb = per.tile([BS, 1], F32, tag="r", name="r")
            with nc.named_scope("softmax"):
                nc.scalar.activation(
                    E_sb[:],
                    ps_s[:],
                    mybir.ActivationFunctionType.Exp,
                    scale=1.0 / 32.0,
                    accum_out=denom[:],
                )
                nc.vector.reciprocal(r_sb[:], denom[:])

            # ---- ET = E.T * prior  -> 2 bf16 tiles [128, BS]
            ET = [
                per.tile([128, BS], BF, tag=f"ET{t}", name=f"ET{t}") for t in range(KT)
            ]
            with nc.named_scope("ET"):
                for t in range(KT):
                    pa = pst.tile([128, 128], BF, tag="pc", name="pa")
                    nc.tensor.transpose(
                        pa[:], E_sb[:, 128 * t : 128 * (t + 1)], ident_bf[:]
                    )
                    nc.vector.tensor_copy(ET[t][:], pa[:])

            # ---- gz = E @ (conf * prior) : psum [BS, D] as two 512-banks
            ND = D // 2  # 512
            psg = [
                ps.tile([BS, ND], F32, tag="bank", name=f"psg{h}") for h in range(2)
            ]
            with nc.named_scope("gz_mm"):
                for t in range(KT):
                    for h in range(2):
                        nc.tensor.matmul(
                            psg[h][:],
                            lhsT=ET[t][:],
                            rhs=confp_sb[:, D * t + ND * h : D * t + ND * (h + 1)],
                            start=(t == 0),
                            stop=(t == KT - 1),
                        )

            # ---- out = gz * (1/denom) + jf ; 4-way split so the fused
            # multiply-add, and the output DMAs pipeline.
            out_sb = [
                per.tile([BS, ND], F32, tag=f"out{h}", name=f"out{h}")
                for h in range(2)
            ]
            NE = ND // 2  # 256
            with nc.named_scope("epilogue"):
                for qtr in range(4):
                    h, j = qtr // 2, qtr % 2
                    nc.vector.scalar_tensor_tensor(
                        out_sb[h][:, NE * j : NE * (j + 1)],
                        psg[h][:, NE * j : NE * (j + 1)],
                        r_sb[:],
                        jf_sb[:, ND * h + NE * j : ND * h + NE * (j + 1)],
                        op0=mybir.AluOpType.mult,
                        op1=mybir.AluOpType.add,
                    )
                    if j == 1:
                        nc.sync.dma_start(
                            out=out.ap()[:, ND * h : ND * (h + 1)],
                            in_=out_sb[h][:],
                        )

    nc.compile()
    if saved_flags:
        set_compiler_flags(saved_flags)
    return nc


def _get_compiled():
    if "nc" not in _COMPILED:
        _COMPILED["nc"] = _build()
    return _COMPILED["nc"]


def kernel(joint_feature, confounder_dictionary, prior, Wq, Wk):
    import ml_dtypes

    from concourse import bass_utils

    nc = _get_compiled()

    bf16 = ml_dtypes.bfloat16
    jf = np.ascontiguousarray(np.asarray(joint_feature, dtype=np.float32))
    conf32 = np.asarray(confounder_dictionary, dtype=np.float32)
    pri = np.asarray(prior, dtype=np.float32)
    conft = np.ascontiguousarray(conf32.T.astype(bf16))
    confp = np.ascontiguousarray((conf32 * pri).astype(bf16))
    wq = np.ascontiguousarray(np.asarray(Wq, dtype=np.float32).astype(bf16))
    wk = np.ascontiguousarray(np.asarray(Wk, dtype=np.float32).astype(bf16))

    in_maps = [
        {
            "jf": jf[i * BS : (i + 1) * BS],
            "conft": conft,
            "confp": confp,
            "wq": wq,
            "wk": wk,
        }
        for i in range(N_CORES)
    ]

    res = bass_utils.run_bass_kernel_spmd(
        nc, in_maps, core_ids=list(range(N_CORES))
    )
    return np.concatenate([res.results[i]["out"] for i in range(N_CORES)], axis=0)


# revision 39
# speedup vs baseline: 1.0832x; 1.0832x over previous
"""Trainium2 Bass kernel for nn_CCIM (dot-product intervention / CCIM block).

Reference computation (B=1024, K=256, D=1024, P=768):
    q = jf @ Wq                      [B, P]
    k = conf @ Wk                    [K, P]
    s = (q @ k.T) / 32               [B, K]
    a = softmax(s, axis=-1)          [B, K]
    out = jf + a @ (conf * prior)    [B, D]

Distribution: data-parallel over B across 8 NeuronCores (128 rows each);
weights/confounders replicated on every core; no collectives.

Matmuls run in bf16 with fp32 PSUM accumulation; jf stays exact fp32 through
to the final add, so bf16 rounding only perturbs the small g_z correction
term (measured rel-L2 vs the fp32 reference: 2.7e-4). The host pre-casts
Wq/Wk to bf16 and pre-computes conf.T (k-chain operand) and conf*prior
(g_z operand) - numerically equivalent to on-device casting/transposing but
saves HBM traffic, all 16 conf PE-transposes, and the prior multiply.

Per-core schedule (engineered against the neuron-profile trace):
  - ~4.5us of dummy matmuls first flip the PE HAM clock-gate to 2.4 GHz.
  - DMA rings are dependency-isolated: Sync ring carries jf then the 8 Wq
    chunks then conf*prior; Scalar ring carries the 8 Wk chunks; the GpSimd
    (SWDGE) ring carries conf.T (its 512B-row packets would otherwise queue
    ahead of Wq). Rings drain in instruction order, so order = priority.
  - jf.T is built with bf16 PE transposes (xbar DMA-transpose serializes all
    DMA traffic and is avoided).
  - kT (=(conf@Wk).T) and qT (=(jf@Wq).T) accumulate over D-chunks with their
    matmuls interleaved per chunk (the PE stream executes strictly in order,
    so phase-ordered emission would head-of-line block).
  - PSUM sub-tiles pack 2-4 accumulation groups per bank: the bank's single
    start=True matmul clears the whole bank's has_written bits, later groups
    first-write with start=False, ordered via add_dep_helper.
  - scores = qT.T @ kT per P-tile; exp(s/32) with fused row-sum on ACT (no
    max-subtraction needed: |s|/32 < ~6); E.T via PE transpose; gz = E.T-
    tiles @ (conf*prior); out = gz*(1/denom) + jf fused on DVE, 4-way split
    and pipelined with the two output DMAs.
"""

import numpy as np

B, K, D, P = 1024, 256, 1024, 768
N_CORES = 8
BS = B // N_CORES  # 128 rows per core

_COMPILED = {}


def _build():
    import concourse.mybir as mybir
    import concourse.tile as tile
    from concourse import bacc
    from concourse.tile_rust import add_dep_helper
    from concourse.compiler_utils import get_compiler_flags, set_compiler_flags
    from concourse.masks import make_identity

    saved_flags = get_compiler_flags()
    if saved_flags:
        set_compiler_flags(
            [
                f.replace("--enable-ldw-opt=false", "--enable-ldw-opt=true")
                for f in saved_flags
            ]
        )

    F32 = mybir.dt.float32
    BF = mybir.dt.bfloat16
    KD = D // 128  # 8 contraction tiles over D
    MP = P // 128  # 6 partition tiles over P
    KT = K // 128  # 2 tiles over K

    nc = bacc.Bacc(
        "TRN2",
        target_bir_lowering=False,
        debug=False,
        num_devices=N_CORES,
    )

    jf = nc.dram_tensor("jf", [BS, D], F32, kind="ExternalInput")
    conft = nc.dram_tensor("conft", [D, K], BF, kind="ExternalInput")
    confp = nc.dram_tensor("confp", [K, D], BF, kind="ExternalInput")
    wq = nc.dram_tensor("wq", [D, P], BF, kind="ExternalInput")
    wk = nc.dram_tensor("wk", [D, P], BF, kind="ExternalInput")
    out = nc.dram_tensor("out", [BS, D], F32, kind="ExternalOutput")

    with tile.TileContext(nc) as tc:
        with (
            tc.tile_pool(name="cst", bufs=1) as cst,
            tc.tile_pool(name="per", bufs=1) as per,
            tc.tile_pool(name="wqp", bufs=1) as wqp,
            tc.tile_pool(name="wkp", bufs=1) as wkp,
            tc.tile_pool(name="ps", bufs=6, space="PSUM") as ps,
            tc.tile_pool(name="pst", bufs=2, space="PSUM") as pst,
        ):
            # Identity first: its gpsimd memset/affine must precede the 16
            # SWDGE trigger instructions on the GpSimd queue, or it lands
            # ~13us late and stalls every transpose behind it.
            ident = cst.tile([128, 128], F32, tag="ident", name="ident")
            make_identity(nc, ident[:])
            ident_bf = cst.tile([128, 128], BF, tag="ident_bf", name="ident_bf")
            nc.vector.tensor_copy(ident_bf[:], ident[:])

            psw = ps.tile([BS, 512], F32, tag="bank", name="psw")
            # PE warmup: ~3us of dummy matmuls to flip the HAM clock-gate to
            # 2.4 GHz before real matmuls arrive (psw's first real use also
            # has start=True, so contents don't matter).
            with nc.named_scope("warmup"):
                for _ in range(42):
                    nc.tensor.matmul(
                        psw[:, 0:128], lhsT=ident_bf[:], rhs=ident_bf[:],
                        start=True, stop=True,
                    )

            wqt2 = [
                wqp.tile([128, 2 * P], BF, tag=f"wq{g}", name=f"wq{g}")
                for g in range(KD // 2)
            ]
            wqt = [wqt2[kk // 2][:, P * (kk % 2) : P * (kk % 2 + 1)] for kk in range(KD)]
            wkt2 = [
                wkp.tile([128, 2 * P], BF, tag=f"wk{g}", name=f"wk{g}")
                for g in range(KD // 2)
            ]
            wkt = [wkt2[kk // 2][:, P * (kk % 2) : P * (kk % 2 + 1)] for kk in range(KD)]
            # Wk chunks on the Scalar HWDGE ring (its own ring so Wk traffic
            # never queues behind jf/conf).
            for kk in range(KD):
                nc.scalar.dma_start(
                    out=wkt[kk], in_=wk.ap()[128 * kk : 128 * (kk + 1), :]
                )

            # ---- input DMAs (jf/conf/prior first, then Wq stream, Wk stream)
            jf_sb = per.tile([BS, D], F32, tag="jf", name="jf")
            nc.sync.dma_start(out=jf_sb[:], in_=jf.ap())
            # conf.T comes pre-transposed from the host: one DMA into 8
            # column-blocks [128, K] each (kills 16 PE transposes + copies).
            confT_big = per.tile([128, KD * K], BF, tag="confT", name="confT")
            nc.gpsimd.dma_start(
                out=confT_big[:].rearrange("p (kk c) -> p kk c", kk=KD),
                in_=conft.ap().rearrange("(kk p) c -> p kk c", p=128),
            )
            confT = [confT_big[:, K * kk : K * (kk + 1)] for kk in range(KD)]
            # Wq chunks on the Sync ring, after jf/confT.
            for kk in range(KD):
                nc.sync.dma_start(
                    out=wqt[kk], in_=wq.ap()[128 * kk : 128 * (kk + 1), :]
                )
            # conf*prior (host-folded) for the gz matmul; only needed at the
            # tail, so it loads after the Wq stream.
            confp_sb = per.tile([128, KT * D], BF, tag="confp", name="confp")
            nc.sync.dma_start(
                out=confp_sb[:].rearrange("p (t c) -> p t c", t=KT),
                in_=confp.ap().rearrange("(t p) c -> p t c", p=128),
            )



            # ---- jfT: bf16 PE transposes (1 cyc/row) from jf_bf
            jf_bf = per.tile([BS, D], BF, tag="jf_bf", name="jf_bf")
            jfT_t = [
                per.tile([128, BS], BF, tag=f"jfT{kk}", name=f"jfT{kk}")
                for kk in range(KD)
            ]
            with nc.named_scope("jfT"):
                nc.vector.tensor_copy(jf_bf[:], jf_sb[:])
                for kk in range(KD):
                    pt = pst.tile([128, 256], BF, tag="pc", name="pt")
                    nc.tensor.transpose(
                        pt[:, 0:128], jf_bf[:, 128 * kk : 128 * (kk + 1)], ident_bf[:]
                    )
                    nc.vector.tensor_copy(jfT_t[kk][:], pt[:, 0:128])
            jfT = [t[:] for t in jfT_t]

            # ---- kT and qT matmuls, interleaved per D-chunk so the PE
            # stream (strict in-order) never head-of-line blocks. qT is
            # computed directly (lhsT=Wq tile, rhs=jfT) so no q transpose
            # chain is needed at the end. Both pack 2 sub-tiles per PSUM
            # bank with an ordered first-write (see kT comment).
            psk = [
                ps.tile([128, 2 * K], F32, tag="bank", name=f"psk{i}")
                for i in range(MP // 2)
            ]
            psqt = [
                ps.tile([128, 4 * BS], F32, tag="bank", name="psqt0"),
                ps.tile([128, 2 * BS], F32, tag="bank", name="psqt1"),
            ]

            def psk_ap(mm):
                return psk[mm // 2][:, K * (mm % 2) : K * (mm % 2 + 1)]

            def psqt_ap(pp):
                b, j = (0, pp) if pp < 4 else (1, pp - 4)
                return psqt[b][:, BS * j : BS * (j + 1)]

            bank_opener = {}
            qt_opener = {}
            with nc.named_scope("qk_mm"):
                for kk in range(KD):
                    for mm in range(MP):
                        inst = nc.tensor.matmul(
                            psk_ap(mm),
                            lhsT=wkt[kk][:, 128 * mm : 128 * (mm + 1)],
                            rhs=confT[kk],
                            start=(kk == 0 and mm % 2 == 0),
                            stop=(kk == KD - 1),
                        )
                        if kk == 0:
                            b = mm // 2
                            if mm % 2 == 0:
                                bank_opener[b] = inst
                            else:
                                add_dep_helper(
                                    inst.ins,
                                    bank_opener[b].ins,
                                    sync=False,
                                    reason="first-write waits on bank open",
                                )
                    for pp in range(MP):
                        b, j = (0, pp) if pp < 4 else (1, pp - 4)
                        inst = nc.tensor.matmul(
                            psqt_ap(pp),
                            lhsT=wqt[kk][:, 128 * pp : 128 * (pp + 1)],
                            rhs=jfT[kk],
                            start=(kk == 0 and j == 0),
                            stop=(kk == KD - 1),
                        )
                        if kk == 0:
                            if j == 0:
                                qt_opener[b] = inst
                            else:
                                add_dep_helper(
                                    inst.ins,
                                    qt_opener[b].ins,
                                    sync=False,
                                    reason="first-write waits on bank open",
                                )

            # ---- qT: psum -> bf16 copies (one per packed bank)
            qT3 = [
                per.tile([128, 4 * BS], BF, tag="qT0", name="qT0"),
                per.tile([128, 2 * BS], BF, tag="qT1", name="qT1"),
            ]
            with nc.named_scope("qT"):
                for b in range(2):
                    nc.vector.tensor_copy(qT3[b][:], psqt[b][:])
            qT = [
                qT3[0][:, BS * pp : BS * (pp + 1)] if pp < 4
                else qT3[1][:, BS * (pp - 4) : BS * (pp - 3)]
                for pp in range(MP)
            ]

            kT3 = [
                per.tile([128, 2 * K], BF, tag=f"kT{b}", name=f"kT{b}")
                for b in range(MP // 2)
            ]
            with nc.named_scope("kT_copy"):
                for b in range(MP // 2):
                    nc.vector.tensor_copy(kT3[b][:], psk[b][:])
            kT = [kT3[mm // 2][:, K * (mm % 2) : K * (mm % 2 + 1)] for mm in range(MP)]

            # ---- scores = q @ k.T (accumulate over P tiles)
            ps_s = ps.tile([BS, 512], F32, tag="bank", name="ps_s")[:, 0:K]
            with nc.named_scope("scores"):
                for pp in range(MP):
                    nc.tensor.matmul(
                        ps_s[:],
                        lhsT=qT[pp],
                        rhs=kT[pp],
                        start=(pp == 0),
                        stop=(pp == MP - 1),
                    )

            # ---- softmax numerator + denominator (no max-subtraction)
            E_sb = per.tile([BS, K], BF, tag="E", name="E")
            denom = per.tile([BS, 1], F32, tag="denom", name="denom")
            r_sb = per.tile([BS, 1], F32, tag="r", name="r")
            d_half = per.tile([BS, 2], F32, tag="d_half", name="d_half")
            with nc.named_scope("softmax"):
                for t in range(KT):
                    nc.scalar.activation(
                        E_sb[:, 128 * t : 128 * (t + 1)],
                        ps_s[:, 128 * t : 128 * (t + 1)],
                        mybir.ActivationFunctionType.Exp,
                        scale=1.0 / 32.0,
                        accum_out=d_half[:, t : t + 1],
                    )
                nc.vector.tensor_add(denom[:], d_half[:, 0:1], d_half[:, 1:2])
                nc.vector.reciprocal(r_sb[:], denom[:])

            # ---- ET = E.T * prior  -> 2 bf16 tiles [128, BS]
            ET = [
                per.tile([128, BS], BF, tag=f"ET{t}", name=f"ET{t}") for t in range(KT)
            ]
            with nc.named_scope("ET"):
                for t in range(KT):
                    pa = pst.tile([128, 128], BF, tag="pc", name="pa")
                    nc.tensor.transpose(
                        pa[:], E_sb[:, 128 * t : 128 * (t + 1)], ident_bf[:]
                    )
                    nc.vector.tensor_copy(ET[t][:], pa[:])

            # ---- gz = E @ (conf * prior) : psum [BS, D] as two 512-banks
            ND = D // 2  # 512
            psg = [
                ps.tile([BS, ND], F32, tag="bank", name=f"psg{h}") for h in range(2)
            ]
            with nc.named_scope("gz_mm"):
                for t in range(KT):
                    for h in range(2):
                        nc.tensor.matmul(
                            psg[h][:],
                            lhsT=ET[t][:],
                            rhs=confp_sb[:, D * t + ND * h : D * t + ND * (h + 1)],
                            start=(t == 0),
                            stop=(t == KT - 1),
                        )

            # ---- out = gz * (1/denom) + jf ; 4-way split so the fused
            # multiply-add, and the output DMAs pipeline.
            out_sb = [
                per.tile([BS, ND], F32, tag=f"out{h}", name=f"out{h}")
                for h in range(2)
            ]
            NE = ND // 2  # 256
            with nc.named_scope("epilogue"):
                for qtr in range(4):
                    h, j = qtr // 2, qtr % 2
                    nc.vector.scalar_tensor_tensor(
                        out_sb[h][:, NE * j : NE * (j + 1)],
                        psg[h][:, NE * j : NE * (j + 1)],
                        r_sb[:],
                        jf_sb[:, ND * h + NE * j : ND * h + NE * (j + 1)],
                        op0=mybir.AluOpType.mult,
                        op1=mybir.AluOpType.add,
                    )
                    if j == 1:
                        nc.sync.dma_start(
                            out=out.ap()[:, ND * h : ND * (h + 1)],
                            in_=out_sb[h][:],
                        )

    nc.compile()
    if saved_flags:
        set_compiler_flags(saved_flags)
    return nc


def _get_compiled():
    if "nc" not in _COMPILED:
        _COMPILED["nc"] = _build()
    return _COMPILED["nc"]


def kernel(joint_feature, confounder_dictionary, prior, Wq, Wk):
    import ml_dtypes

    from concourse import bass_utils

    nc = _get_compiled()

    bf16 = ml_dtypes.bfloat16
    jf = np.ascontiguousarray(np.asarray(joint_feature, dtype=np.float32))
    conf32 = np.asarray(confounder_dictionary, dtype=np.float32)
    pri = np.asarray(prior, dtype=np.float32)
    conft = np.ascontiguousarray(conf32.T.astype(bf16))
    confp = np.ascontiguousarray((conf32 * pri).astype(bf16))
    wq = np.ascontiguousarray(np.asarray(Wq, dtype=np.float32).astype(bf16))
    wk = np.ascontiguousarray(np.asarray(Wk, dtype=np.float32).astype(bf16))

    in_maps = [
        {
            "jf": jf[i * BS : (i + 1) * BS],
            "conft": conft,
            "confp": confp,
            "wq": wq,
            "wk": wk,
        }
        for i in range(N_CORES)
    ]

    res = bass_utils.run_bass_kernel_spmd(
        nc, in_maps, core_ids=list(range(N_CORES))
    )
    return np.concatenate([res.results[i]["out"] for i in range(N_CORES)], axis=0)


# revision 42
# speedup vs baseline: 1.1054x; 1.0204x over previous
"""Trainium2 Bass kernel for nn_CCIM (dot-product intervention / CCIM block).

Reference computation (B=1024, K=256, D=1024, P=768):
    q = jf @ Wq                      [B, P]
    k = conf @ Wk                    [K, P]
    s = (q @ k.T) / 32               [B, K]
    a = softmax(s, axis=-1)          [B, K]
    out = jf + a @ (conf * prior)    [B, D]

Distribution: data-parallel over B across 8 NeuronCores (128 rows each);
weights/confounders replicated on every core; no collectives.

Matmuls run in bf16 with fp32 PSUM accumulation; jf stays exact fp32 through
to the final add, so bf16 rounding only perturbs the small g_z correction
term (measured rel-L2 vs the fp32 reference: 2.7e-4). The host pre-casts
Wq/Wk to bf16 and pre-computes conf.T (k-chain operand) and conf*prior
(g_z operand) - numerically equivalent to on-device casting/transposing but
saves HBM traffic, all 16 conf PE-transposes, and the prior multiply.

Per-core schedule (engineered against the neuron-profile trace):
  - ~4.5us of dummy matmuls first flip the PE HAM clock-gate to 2.4 GHz.
  - DMA rings are dependency-isolated: Sync ring carries jf then the 8 Wq
    chunks then conf*prior; Scalar ring carries the 8 Wk chunks; the GpSimd
    (SWDGE) ring carries conf.T (its 512B-row packets would otherwise queue
    ahead of Wq). Rings drain in instruction order, so order = priority.
  - jf.T is built with bf16 PE transposes (xbar DMA-transpose serializes all
    DMA traffic and is avoided).
  - kT (=(conf@Wk).T) and qT (=(jf@Wq).T) accumulate over D-chunks with their
    matmuls interleaved per chunk (the PE stream executes strictly in order,
    so phase-ordered emission would head-of-line block).
  - PSUM sub-tiles pack 2-4 accumulation groups per bank: the bank's single
    start=True matmul clears the whole bank's has_written bits, later groups
    first-write with start=False, ordered via add_dep_helper.
  - scores = qT.T @ kT per P-tile; exp(s/32) with fused row-sum on ACT (no
    max-subtraction needed: |s|/32 < ~6); E.T via PE transpose; gz = E.T-
    tiles @ (conf*prior); out = gz*(1/denom) + jf fused on DVE, 4-way split
    and pipelined with the two output DMAs.
"""

import numpy as np

B, K, D, P = 1024, 256, 1024, 768
N_CORES = 8
BS = B // N_CORES  # 128 rows per core

_COMPILED = {}


def _build():
    import concourse.mybir as mybir
    import concourse.tile as tile
    from concourse import bacc
    from concourse.tile_rust import add_dep_helper
    from concourse.compiler_utils import get_compiler_flags, set_compiler_flags
    from concourse.masks import make_identity

    saved_flags = get_compiler_flags()
    if saved_flags:
        set_compiler_flags(
            [
                f.replace("--enable-ldw-opt=false", "--enable-ldw-opt=true")
                for f in saved_flags
            ]
        )

    F32 = mybir.dt.float32
    BF = mybir.dt.bfloat16
    KD = D // 128  # 8 contraction tiles over D
    MP = P // 128  # 6 partition tiles over P
    KT = K // 128  # 2 tiles over K

    nc = bacc.Bacc(
        "TRN2",
        target_bir_lowering=False,
        debug=False,
        num_devices=N_CORES,
    )

    jf = nc.dram_tensor("jf", [BS, D], F32, kind="ExternalInput")
    conft = nc.dram_tensor("conft", [D, K], BF, kind="ExternalInput")
    confp = nc.dram_tensor("confp", [K, D], BF, kind="ExternalInput")
    wq = nc.dram_tensor("wq", [D, P], BF, kind="ExternalInput")
    wk = nc.dram_tensor("wk", [D, P], BF, kind="ExternalInput")
    out = nc.dram_tensor("out", [BS, D], F32, kind="ExternalOutput")

    with tile.TileContext(nc) as tc:
        with (
            tc.tile_pool(name="cst", bufs=1) as cst,
            tc.tile_pool(name="per", bufs=1) as per,
            tc.tile_pool(name="wqp", bufs=1) as wqp,
            tc.tile_pool(name="wkp", bufs=1) as wkp,
            tc.tile_pool(name="ps", bufs=6, space="PSUM") as ps,
            tc.tile_pool(name="pst", bufs=2, space="PSUM") as pst,
        ):
            # Identity first: its gpsimd memset/affine must precede the 16
            # SWDGE trigger instructions on the GpSimd queue, or it lands
            # ~13us late and stalls every transpose behind it.
            ident = cst.tile([128, 128], F32, tag="ident", name="ident")
            make_identity(nc, ident[:])
            ident_bf = cst.tile([128, 128], BF, tag="ident_bf", name="ident_bf")
            nc.vector.tensor_copy(ident_bf[:], ident[:])

            psw = ps.tile([BS, 512], F32, tag="bank", name="psw")
            # PE warmup: ~3us of dummy matmuls to flip the HAM clock-gate to
            # 2.4 GHz before real matmuls arrive (psw's first real use also
            # has start=True, so contents don't matter).
            with nc.named_scope("warmup"):
                for _ in range(42):
                    nc.tensor.matmul(
                        psw[:, 0:128], lhsT=ident_bf[:], rhs=ident_bf[:],
                        start=True, stop=True,
                    )

            wqt2 = [
                wqp.tile([128, 2 * P], BF, tag=f"wq{g}", name=f"wq{g}")
                for g in range(KD // 2)
            ]
            wqt = [wqt2[kk // 2][:, P * (kk % 2) : P * (kk % 2 + 1)] for kk in range(KD)]
            wkt2 = [
                wkp.tile([128, 2 * P], BF, tag=f"wk{g}", name=f"wk{g}")
                for g in range(KD // 2)
            ]
            wkt = [wkt2[kk // 2][:, P * (kk % 2) : P * (kk % 2 + 1)] for kk in range(KD)]
            # Wk chunks on the Scalar HWDGE ring (its own ring so Wk traffic
            # never queues behind jf/conf).
            for kk in range(KD):
                nc.scalar.dma_start(
                    out=wkt[kk], in_=wk.ap()[128 * kk : 128 * (kk + 1), :]
                )

            # ---- input DMAs (jf/conf/prior first, then Wq stream, Wk stream)
            jf_sb = per.tile([BS, D], F32, tag="jf", name="jf")
            nc.sync.dma_start(out=jf_sb[:], in_=jf.ap())
            # conf.T comes pre-transposed from the host: one DMA into 8
            # column-blocks [128, K] each (kills 16 PE transposes + copies).
            confT_big = per.tile([128, KD * K], BF, tag="confT", name="confT")
            nc.gpsimd.dma_start(
                out=confT_big[:].rearrange("p (kk c) -> p kk c", kk=KD),
                in_=conft.ap().rearrange("(kk p) c -> p kk c", p=128),
            )
            confT = [confT_big[:, K * kk : K * (kk + 1)] for kk in range(KD)]
            # Wq chunks on the Sync ring, after jf/confT.
            for kk in range(KD):
                nc.sync.dma_start(
                    out=wqt[kk], in_=wq.ap()[128 * kk : 128 * (kk + 1), :]
                )
            # conf*prior (host-folded) for the gz matmul; only needed at the
            # tail, so it loads after the Wq stream.
            confp_sb = per.tile([128, KT * D], BF, tag="confp", name="confp")
            nc.sync.dma_start(
                out=confp_sb[:].rearrange("p (t c) -> p t c", t=KT),
                in_=confp.ap().rearrange("(t p) c -> p t c", p=128),
            )



            # ---- jfT: bf16 PE transposes (1 cyc/row) from jf_bf
            jf_bf = per.tile([BS, D], BF, tag="jf_bf", name="jf_bf")
            jfT_t = [
                per.tile([128, BS], BF, tag=f"jfT{kk}", name=f"jfT{kk}")
                for kk in range(KD)
            ]
            with nc.named_scope("jfT"):
                nc.vector.tensor_copy(jf_bf[:], jf_sb[:])
                for kk in range(KD):
                    pt = pst.tile([128, 256], BF, tag="pc", name="pt")
                    nc.tensor.transpose(
                        pt[:, 0:128], jf_bf[:, 128 * kk : 128 * (kk + 1)], ident_bf[:]
                    )
                    nc.vector.tensor_copy(jfT_t[kk][:], pt[:, 0:128])
            jfT = [t[:] for t in jfT_t]

            # ---- kT and qT matmuls, interleaved per D-chunk so the PE
            # stream (strict in-order) never head-of-line blocks. qT is
            # computed directly (lhsT=Wq tile, rhs=jfT) so no q transpose
            # chain is needed at the end. Both pack 2 sub-tiles per PSUM
            # bank with an ordered first-write (see kT comment).
            psk = [
                ps.tile([128, 2 * K], F32, tag="bank", name=f"psk{i}")
                for i in range(MP // 2)
            ]
            psqt = [
                ps.tile([128, 4 * BS], F32, tag="bank", name="psqt0"),
                ps.tile([128, 2 * BS], F32, tag="bank", name="psqt1"),
            ]

            def psk_ap(mm):
                return psk[mm // 2][:, K * (mm % 2) : K * (mm % 2 + 1)]

            def psqt_ap(pp):
                b, j = (0, pp) if pp < 4 else (1, pp - 4)
                return psqt[b][:, BS * j : BS * (j + 1)]

            bank_opener = {}
            qt_opener = {}
            with nc.named_scope("qk_mm"):
                for kk in range(KD):
                    for mm in range(MP):
                        inst = nc.tensor.matmul(
                            psk_ap(mm),
                            lhsT=wkt[kk][:, 128 * mm : 128 * (mm + 1)],
                            rhs=confT[kk],
                            start=(kk == 0 and mm % 2 == 0),
                            stop=(kk == KD - 1),
                        )
                        if kk == 0:
                            b = mm // 2
                            if mm % 2 == 0:
                                bank_opener[b] = inst
                            else:
                                add_dep_helper(
                                    inst.ins,
                                    bank_opener[b].ins,
                                    sync=False,
                                    reason="first-write waits on bank open",
                                )
                    for pp in range(MP):
                        b, j = (0, pp) if pp < 4 else (1, pp - 4)
                        inst = nc.tensor.matmul(
                            psqt_ap(pp),
                            lhsT=wqt[kk][:, 128 * pp : 128 * (pp + 1)],
                            rhs=jfT[kk],
                            start=(kk == 0 and j == 0),
                            stop=(kk == KD - 1),
                        )
                        if kk == 0:
                            if j == 0:
                                qt_opener[b] = inst
                            else:
                                add_dep_helper(
                                    inst.ins,
                                    qt_opener[b].ins,
                                    sync=False,
                                    reason="first-write waits on bank open",
                                )

            # ---- qT: psum -> bf16 copies (one per packed bank)
            qT3 = [
                per.tile([128, 4 * BS], BF, tag="qT0", name="qT0"),
                per.tile([128, 2 * BS], BF, tag="qT1", name="qT1"),
            ]
            with nc.named_scope("qT"):
                for b in range(2):
                    nc.vector.tensor_copy(qT3[b][:], psqt[b][:])
            qT = [
                qT3[0][:, BS * pp : BS * (pp + 1)] if pp < 4
                else qT3[1][:, BS * (pp - 4) : BS * (pp - 3)]
                for pp in range(MP)
            ]

            kT3 = [
                per.tile([128, 2 * K], BF, tag=f"kT{b}", name=f"kT{b}")
                for b in range(MP // 2)
            ]
            with nc.named_scope("kT_copy"):
                for b in range(MP // 2):
                    nc.vector.tensor_copy(kT3[b][:], psk[b][:])
            kT = [kT3[mm // 2][:, K * (mm % 2) : K * (mm % 2 + 1)] for mm in range(MP)]

            # ---- scores = q @ k.T (accumulate over P tiles)
            ps_s = ps.tile([BS, 512], F32, tag="bank", name="ps_s")[:, 0:K]
            with nc.named_scope("scores"):
                for pp in range(MP):
                    nc.tensor.matmul(
                        ps_s[:],
                        lhsT=qT[pp],
                        rhs=kT[pp],
                        start=(pp == 0),
                        stop=(pp == MP - 1),
                    )

            # ---- softmax numerator + denominator (no max-subtraction)
            E_sb = per.tile([BS, K], BF, tag="E", name="E")
            denom = per.tile([BS, 1], F32, tag="denom", name="denom")
            r_sb = per.tile([BS, 1], F32, tag="r", name="r")
            d_half = per.tile([BS, 2], F32, tag="d_half", name="d_half")
            with nc.named_scope("softmax"):
                for t in range(KT):
                    nc.scalar.activation(
                        E_sb[:, 128 * t : 128 * (t + 1)],
                        ps_s[:, 128 * t : 128 * (t + 1)],
                        mybir.ActivationFunctionType.Exp,
                        scale=1.0 / 32.0,
                        accum_out=d_half[:, t : t + 1],
                    )
                nc.vector.tensor_add(denom[:], d_half[:, 0:1], d_half[:, 1:2])
                nc.vector.reciprocal(r_sb[:], denom[:])

            # ---- ET = E.T * prior  -> 2 bf16 tiles [128, BS]
            ET = [
                per.tile([128, BS], BF, tag=f"ET{t}", name=f"ET{t}") for t in range(KT)
            ]
            with nc.named_scope("ET"):
                for t in range(KT):
                    pa = pst.tile([128, 128], BF, tag="pc", name="pa")
                    nc.tensor.transpose(
                        pa[:], E_sb[:, 128 * t : 128 * (t + 1)], ident_bf[:]
                    )
                    nc.vector.tensor_copy(ET[t][:], pa[:])

            # ---- gz = E @ (conf * prior) : psum [BS, D] as two 512-banks
            ND = D // 2  # 512
            psg = [
                ps.tile([BS, ND], F32, tag="bank", name=f"psg{h}") for h in range(2)
            ]
            with nc.named_scope("gz_mm"):
                for t in range(KT):
                    for h in range(2):
                        nc.tensor.matmul(
                            psg[h][:],
                            lhsT=ET[t][:],
                            rhs=confp_sb[:, D * t + ND * h : D * t + ND * (h + 1)],
                            start=(t == 0),
                            stop=(t == KT - 1),
                        )

            # ---- out = gz * (1/denom) + jf ; 4-way split so the fused
            # multiply-add, and the output DMAs pipeline.
            out_sb = [
                per.tile([BS, ND], F32, tag=f"out{h}", name=f"out{h}")
                for h in range(2)
            ]
            NE = ND // 2  # 256
            with nc.named_scope("epilogue"):
                for qtr in range(4):
                    h, j = qtr // 2, qtr % 2
                    nc.vector.scalar_tensor_tensor(
                        out_sb[h][:, NE * j : NE * (j + 1)],
                        psg[h][:, NE * j : NE * (j + 1)],
                        r_sb[:],
                        jf_sb[:, ND * h + NE * j : ND * h + NE * (j + 1)],
                        op0=mybir.AluOpType.mult,
                        op1=mybir.AluOpType.add,
                    )
                    if j == 1:
                        nc.sync.dma_start(
                            out=out.ap()[:, ND * h : ND * (h + 1)],
                            in_=out_sb[h][:],
                        )

    nc.compile()
    if saved_flags:
        set_compiler_flags(saved_flags)
    return nc


def _get_compiled():
    if "nc" not in _COMPILED:
        _COMPILED["nc"] = _build()
    return _COMPILED["nc"]


def kernel(joint_feature, confounder_dictionary, prior, Wq, Wk):
    import ml_dtypes

    from concourse import bass_utils

    nc = _get_compiled()

    bf16 = ml_dtypes.bfloat16
    jf = np.ascontiguousarray(np.asarray(joint_feature, dtype=np.float32))
    conf32 = np.asarray(confounder_dictionary, dtype=np.float32)
    pri = np.asarray(prior, dtype=np.float32)
    conft = np.ascontiguousarray(conf32.T.astype(bf16))
    confp = np.ascontiguousarray((conf32 * pri).astype(bf16))
    wq = np.ascontiguousarray(np.asarray(Wq, dtype=np.float32).astype(bf16))
    wk = np.ascontiguousarray(np.asarray(Wk, dtype=np.float32).astype(bf16))

    in_maps = [
        {
            "jf": jf[i * BS : (i + 1) * BS],
            "conft": conft,
            "confp": confp,
            "wq": wq,
            "wk": wk,
        }
        for i in range(N_CORES)
    ]

    res = bass_utils.run_bass_kernel_spmd(
        nc, in_maps, core_ids=list(range(N_CORES))
    )
    return np.concatenate([res.results[i]["out"] for i in range(N_CORES)], axis=0)
